# revision 3
# baseline (speedup 1.0000x reference)
"""Masked-loss kernel for nn_MLoss_9715216024200 on 8 Trainium2 NeuronCores.

loss = sum(where(y[...,0]>0.5, (y-x)^2 - a*x^2, 0)) + a*sum(x[...,0]^2)
with x,y f32 (256, 10647, 5); output is a f32 scalar.

Sharding: flatten both tensors to cells (5 contiguous f32 each), pad with
256 zero-cells (mathematically neutral: y0=0 -> mask 0, x=0 -> no bg term),
reshape to (8 cores, 128 partitions, 2662 cells).  Each core streams its
13 MiB at the 360 B/ns DMA roofline; the schedule is built so every engine
runs below the DMA rate and the post-stream tail is a minimal chain:

  per tile (c cells, fd=5c elems/partition, all intermediates bf16):
    DVE:  m5  = bf16(y0 > 0.5) replicated to 5 features   (2x tensor_scalar)
          d   = y - x                                     (1x)
          dm  = d * m5 -> dmx[0:fd]                       (2x)
    Pool: xs0 = sqrt(a)*x0 -> dmx[fd:fd+c]
          xm  = x * m5
    ACT:  acc[2t]   = sum(dmx^2)  = sum((m*d)^2) + a*sum(x0^2)
          acc[2t+1] = sum((sqrt(a)*xm)^2) = a*sum((m*x)^2)

Tile sizes: a small head tile so compute starts ~3us earlier, then big
steady-state tiles, then a geometrically shrinking tail so that when the
last input DMA lands only a tiny chain remains; the last tiles' squares run
on DVE (scalar_tensor_tensor accum) to avoid cross-engine handoffs.
Host combines: total = sum(acc[even]) - sum(acc[odd]) in f64.
"""
import sys

for _p in ('/opt/trn_rl_repo',):
    if _p in sys.path:
        sys.path.remove(_p)
    sys.path.insert(0, _p)

import os as _os
import numpy as np

B, C, F = 256, 10647, 5
THRESH = 0.5
ALPHA = 0.1
N_CORES = 8
P = 128
CELLS = B * C                      # 2,725,632
CELLS_PER_PART = 2662              # ceil to 8*128*2662 = 2,725,888
PAD_CELLS = N_CORES * P * CELLS_PER_PART - CELLS   # 256
FD = CELLS_PER_PART * F            # 13310 elems per partition per core

_ts = _os.environ.get('TILE_SIZES', '')
TILE_SIZES = ([int(v) for v in _ts.split(',')] if _ts
              else [26, 290, 290, 290, 290, 290, 290, 290, 290, 160, 80, 50, 26])
assert sum(TILE_SIZES) == CELLS_PER_PART, sum(TILE_SIZES)
N_TILES = len(TILE_SIZES)

# engine knobs (tile indices)
_env = _os.environ.get
XM_ON_DVE = set(int(v) for v in _env('XM_ON_DVE', '').split(',') if v != '')
M5_ON_POOL = set(int(v) for v in _env('M5_ON_POOL', '').split(',') if v != '')
SQ_ON_DVE = set(int(v) for v in _env('SQ_ON_DVE', str(N_TILES - 1)).split(',') if v != '')
SQ2_ON_DVE = set(int(v) for v in _env('SQ2_ON_DVE', str(N_TILES - 1)).split(',') if v != '')
BUFS = [int(v) for v in _env('BUFS', '4,4,4,4').split(',')]

_compiled = None


def _build():
    from contextlib import ExitStack
    import concourse.tile as tile
    from concourse import bacc, mybir

    sqa = float(np.sqrt(ALPHA))

    nc = bacc.Bacc("TRN2", target_bir_lowering=False, debug=False,
                   enable_asserts=True, num_devices=N_CORES)
    x_d = nc.dram_tensor("x", [P, FD], mybir.dt.float32, kind="ExternalInput").ap()
    y_d = nc.dram_tensor("y", [P, FD], mybir.dt.float32, kind="ExternalInput").ap()
    o_d = nc.dram_tensor("o", [P, 2 * N_TILES], mybir.dt.float32,
                         kind="ExternalOutput").ap()

    f32 = mybir.dt.float32
    bf16 = mybir.dt.bfloat16
    Sq = mybir.ActivationFunctionType.Square
    Alu = mybir.AluOpType

    with tile.TileContext(nc) as tc, ExitStack() as ctx:
        xp = ctx.enter_context(tc.tile_pool(name="x", bufs=BUFS[0]))
        yp = ctx.enter_context(tc.tile_pool(name="y", bufs=BUFS[1]))
        wp = ctx.enter_context(tc.tile_pool(name="work", bufs=BUFS[2]))
        sp = ctx.enter_context(tc.tile_pool(name="scratch", bufs=BUFS[3]))
        ap_ = ctx.enter_context(tc.tile_pool(name="acc", bufs=1))

        # interleaved acc layout: columns [2t, 2t+1] = (dm-side, xm-side)
        acc = ap_.tile([P, 2 * N_TILES], f32)

        off = 0
        for t, cells in enumerate(TILE_SIZES):
            fd = cells * F
            xt = xp.tile([P, fd], f32, tag="xt")
            yt = yp.tile([P, fd], f32, tag="yt")
            sl = slice(off, off + fd)
            off += fd
            nc.sync.dma_start(yt[:], y_d[:, sl])
            nc.sync.dma_start(xt[:], x_d[:, sl])

            dmx = wp.tile([P, fd + cells], bf16, tag="dmx")

            # mask replicated to all 5 features, bf16 (2x tensor_scalar on DVE)
            m5 = wp.tile([P, fd], bf16, tag="m5")
            y0b = yt[:, 0::F].unsqueeze(2).broadcast_to((P, cells, F))
            m5_eng = nc.gpsimd if t in M5_ON_POOL else nc.vector
            m5_eng.tensor_scalar(
                m5[:].rearrange("p (k f) -> p k f", f=F), y0b,
                THRESH, None, op0=Alu.is_gt)

            # Pool: xs0 = sqrt(a)*x0 into the tail slice of dmx
            nc.gpsimd.tensor_scalar(dmx[:, fd:fd + cells], xt[:, 0::F],
                                    sqa, None, op0=Alu.mult)

            # DVE: d = y - x (bf16 out), dm = d*m5 (2x)
            dt_ = wp.tile([P, fd], bf16, tag="d")
            nc.vector.tensor_tensor(dt_[:], yt[:], xt[:], op=Alu.subtract)
            nc.vector.tensor_tensor(dmx[:, 0:fd], dt_[:], m5[:], op=Alu.mult)

            # xm = x * m5 (Pool by default; DVE is the fallback)
            xmt = wp.tile([P, fd], bf16, tag="xm")
            xm_eng = nc.vector if t in XM_ON_DVE else nc.gpsimd
            xm_eng.tensor_tensor(xmt[:], xt[:], m5[:], op=Alu.mult)

            # squares + row-sum into the per-tile accumulator pair
            if t in SQ_ON_DVE:
                sq = sp.tile([P, fd + cells], bf16, tag="sq")
                nc.vector.scalar_tensor_tensor(
                    sq[:], dmx[:], 1.0, dmx[:], op0=Alu.mult, op1=Alu.mult,
                    accum_out=acc[:, 2 * t:2 * t + 1])
            else:
                sq = sp.tile([P, fd + cells], bf16, tag="sq")
                nc.scalar.activation(sq[:], dmx[:], Sq,
                                     accum_out=acc[:, 2 * t:2 * t + 1])
            if t in SQ2_ON_DVE:
                sq2 = sp.tile([P, fd], bf16, tag="sq2")
                nc.vector.scalar_tensor_tensor(
                    sq2[:], xmt[:], ALPHA, xmt[:], op0=Alu.mult, op1=Alu.mult,
                    accum_out=acc[:, 2 * t + 1:2 * t + 2])
            else:
                sq2 = sp.tile([P, fd], bf16, tag="sq2")
                nc.scalar.activation(sq2[:], xmt[:], Sq, scale=sqa,
                                     accum_out=acc[:, 2 * t + 1:2 * t + 2])

        nc.sync.dma_start(o_d[:], acc[:])

    nc.compile()
    return nc


def _shard(a: np.ndarray) -> list[np.ndarray]:
    flat = a.reshape(-1)
    pad = np.zeros(PAD_CELLS * F, dtype=a.dtype)
    flat = np.concatenate([flat, pad])
    per_core = flat.reshape(N_CORES, P, FD)
    return [np.ascontiguousarray(per_core[i]) for i in range(N_CORES)]


def kernel(x: np.ndarray, y: np.ndarray) -> np.ndarray:
    global _compiled
    if _compiled is None:
        _compiled = _build()
    nc = _compiled

    from concourse.bass_utils import run_bass_kernel_spmd

    xs = _shard(np.asarray(x, dtype=np.float32))
    ys = _shard(np.asarray(y, dtype=np.float32))
    in_maps = [{"x": xs[i], "y": ys[i]} for i in range(N_CORES)]
    res = run_bass_kernel_spmd(nc, in_maps, core_ids=list(range(N_CORES)))

    total = np.float64(0.0)
    for r in res.results:
        o = r["o"].astype(np.float64)
        total += o[:, 0::2].sum()
        total -= o[:, 1::2].sum()
    return np.float32(total)
